# revision 1
# baseline (speedup 1.0000x reference)
"""Trainium2 Bass kernel for nn_CommNetActor.

Network (per sample, 4 agents, all weights shared across agents):
    H0 = sigmoid(O @ enc_w + enc_b)            [B,4,128]
    H1..H3 = relu chain of 128x128 fc layers
    C  = (sum_j H3[:,j] - H3) / 4              (CommNet neighbour mean)
    H4 = [H3 | C] @ cl4_w + cl4_b
    logits = H4.reshape(B,512) @ dec_w + dec_b
    out = softmax(logits)                      [B,16]

Key algebraic fold used here: since C is linear in the H3 agent slices,
the whole tail (neighbour mean + cl4 + dec) collapses into per-agent
readout matrices applied directly to H3:
    logits[b] = sum_a H3[b,a] @ Wz_a + bias'
    Wz_a  = cl4_w[:128] @ D_a + 0.25 * cl4_w[128:] @ (sum_j D_j - D_a)
    bias' = dec_b + cl4_b @ sum_j D_j,      D_a = dec_w[128a:128a+128]
This removes ~35% of the FLOPs, the cross-agent reduction, and the
concat entirely.

Sigmoid is rewritten as tanh so every ScalarE function used (tanh,
relu, exp) lives in one activation-table set:
    sigmoid(x) = 0.5 + 0.5 tanh(x/2)
    H0 := tanh(0.5 x + 0.5 enc_b);  fc1 folded: W1' = 0.5 W1,
    b1' = fc1_b + 0.5 colsum(fc1_w)

Layout: pure data parallelism over 8 cores (8192 samples each). All
activations live transposed in SBUF as [feature(=partition), column],
columns agent-planar per 1024-sample super-tile; sample s of agent a
sits at column (s//512)*2048 + a*512 + s%512. The input is
pre-transposed on the host (no on-device transpose), packed two
samples per column ([128, 2048] per super-tile) so the input DMA uses
all 128 partitions and the K=64 enc matmul runs as two concurrent
row-group-tiled matmuls. Trunk matmuls use float32r (full fp32
storage, 1 cycle/row PE path, measured ~1e-4 rel err end-to-end).
The readout runs activation-stationary (lhsT = H3 chunk in bf16 so
fast-weight-load applies), producing logits in natural [sample, class]
orientation, so softmax is a plain free-dim reduction. ScalarE uses
only {tanh, relu, exp} = one activation-table set (sigmoid was
rewritten as tanh with the affine folded into fc1's weights).
"""

import numpy as np

import concourse.bass as bass
import concourse.mybir as mybir
import concourse.tile as tile
from concourse import bacc
from concourse.bass import ts
from concourse.bass_utils import run_bass_kernel_spmd

# ---- problem constants (hardcoded per the task contract) ----
B = 65536
A = 4
OBS = 64
D = 128
C = 16
NCORES = 8
BLOC = B // NCORES          # samples per core
ST = 1024                   # samples per super-tile
NST = BLOC // ST
COLS = A * ST               # transposed columns per super-tile
NCHUNK = 512                # matmul moving-dim chunk (one f32 PSUM bank)
GROUPS = ST // D            # 128-sample readout chunks per super-tile

F32 = mybir.dt.float32
F32R = mybir.dt.float32r    # full fp32 storage, fast PE path
BF16 = mybir.dt.bfloat16
AFT = mybir.ActivationFunctionType
ALU = mybir.AluOpType

# matmul input dtype for the main trunk: F32R (fast, ~fp32 storage) or
# F32 (4x slower PE, bit-accurate) or BF16.
TRUNK_DT = F32R

_compiled = {}


def _build_bass():
    # Bacc (not plain Bass): its compile() runs generate_event_semaphores /
    # move_matmul_waits_to_ldweights, which legalize multi-wait instructions
    # down to the TRN2 limit of one sync wait per instruction.
    nc = bacc.Bacc()

    # Input packed two-samples-per-column: partitions 0-63 hold features of
    # the first half of each super-tile's samples, 64-127 the second half.
    # Full 128-partition DMA + the enc matmul runs as two concurrent
    # row-group-tiled K=64 matmuls (tile_position (0,0) / (64,0)).
    ot_d = nc.dram_tensor("ot", [2 * OBS, NST, COLS // 2], TRUNK_DT, kind="ExternalInput")
    ew_d = nc.dram_tensor("enc_w", [2 * OBS, D], TRUNK_DT, kind="ExternalInput")
    w1_d = nc.dram_tensor("w1", [D, D], TRUNK_DT, kind="ExternalInput")
    w2_d = nc.dram_tensor("w2", [D, D], TRUNK_DT, kind="ExternalInput")
    w3_d = nc.dram_tensor("w3", [D, D], TRUNK_DT, kind="ExternalInput")
    wz_d = nc.dram_tensor("wz", [D, A * C], BF16, kind="ExternalInput")
    eb_d = nc.dram_tensor("eb", [D, GROUPS * C], F32, kind="ExternalInput")
    b0_d = nc.dram_tensor("b0", [D, 1], F32, kind="ExternalInput")
    b1_d = nc.dram_tensor("b1", [D, 1], F32, kind="ExternalInput")
    b2_d = nc.dram_tensor("b2", [D, 1], F32, kind="ExternalInput")
    b3_d = nc.dram_tensor("b3", [D, 1], F32, kind="ExternalInput")
    out_d = nc.dram_tensor("probs", [BLOC, C], F32, kind="ExternalOutput")

    with tile.TileContext(nc) as tc:
        with (
            tc.tile_pool(name="consts", bufs=1) as cpool,
            tc.tile_pool(name="ot", bufs=2) as opool,
            tc.tile_pool(name="acts", bufs=2) as hpool,
            tc.tile_pool(name="soft", bufs=2) as spool,
            tc.tile_pool(name="mm", bufs=3, space="PSUM") as mmpool,
            tc.tile_pool(name="lg", bufs=2, space="PSUM") as lgpool,
        ):
            ew_t = cpool.tile([2 * OBS, D], TRUNK_DT, name="ew")
            nc.sync.dma_start(ew_t[:], ew_d[:])
            w_t = {}
            for nm, dd in (("w1", w1_d), ("w2", w2_d), ("w3", w3_d)):
                w_t[nm] = cpool.tile([D, D], TRUNK_DT, name=nm)
                nc.sync.dma_start(w_t[nm][:], dd[:])
            wz_t = cpool.tile([D, A * C], BF16, name="wz")
            nc.sync.dma_start(wz_t[:], wz_d[:])
            eb_t = cpool.tile([D, GROUPS * C], F32, name="eb")
            nc.sync.dma_start(eb_t[:], eb_d[:])
            b_t = {}
            for nm, dd in (("b0", b0_d), ("b1", b1_d), ("b2", b2_d), ("b3", b3_d)):
                b_t[nm] = cpool.tile([D, 1], F32, name=nm)
                nc.sync.dma_start(b_t[nm][:], dd[:])

            for st in range(NST):
                # ---- input: [128, 2048] two-half packed ----
                ot_t = opool.tile([2 * OBS, COLS // 2], TRUNK_DT, tag="ot")
                nc.sync.dma_start(ot_t[:], ot_d[:, st, :])

                # ---- enc: tanh(0.5 x + 0.5 b); ACT engine ----
                # j interleaves halves (0,2,1,3) so consecutive matmuls sit in
                # different PE row groups and execute concurrently.
                h0 = hpool.tile([D, COLS], TRUNK_DT, tag="h0")
                for j in (0, 2, 1, 3):
                    hh = j // 2
                    base = (j % 2) * 1024
                    ps = mmpool.tile([D, 1024], F32, tag="mm")
                    for k in range(2):
                        nc.tensor.matmul(
                            ps[:, ts(k, NCHUNK)],
                            ew_t[64 * hh : 64 * (hh + 1), :],
                            ot_t[64 * hh : 64 * (hh + 1),
                                 base + k * NCHUNK : base + (k + 1) * NCHUNK],
                            start=True, stop=True,
                        )
                    nc.scalar.activation(
                        h0[:, ts(j, 1024)], ps[:], AFT.Tanh,
                        bias=b_t["b0"][:], scale=0.5,
                    )

                # ---- fc1 relu: DVE (bias-add + max0 fused) ----
                h1 = hpool.tile([D, COLS], TRUNK_DT, tag="h1")
                for j in range(COLS // 1024):
                    ps = mmpool.tile([D, 1024], F32, tag="mm")
                    for k in range(2):
                        nc.tensor.matmul(
                            ps[:, ts(k, NCHUNK)],
                            w_t["w1"][:],
                            h0[:, j * 1024 + k * NCHUNK : j * 1024 + (k + 1) * NCHUNK],
                            start=True, stop=True,
                        )
                    nc.vector.tensor_scalar(
                        h1[:, ts(j, 1024)], ps[:],
                        b_t["b1"][:], 0.0, ALU.add, ALU.max,
                    )

                # ---- fc2 relu: ACT ----
                h2 = hpool.tile([D, COLS], TRUNK_DT, tag="h2")
                for j in range(COLS // 1024):
                    ps = mmpool.tile([D, 1024], F32, tag="mm")
                    for k in range(2):
                        nc.tensor.matmul(
                            ps[:, ts(k, NCHUNK)],
                            w_t["w2"][:],
                            h1[:, j * 1024 + k * NCHUNK : j * 1024 + (k + 1) * NCHUNK],
                            start=True, stop=True,
                        )
                    nc.scalar.activation(
                        h2[:, ts(j, 1024)], ps[:], AFT.Relu, bias=b_t["b2"][:],
                    )

                # ---- fc3 relu -> bf16 H3 (readout operand); split ACT/DVE ----
                h3 = hpool.tile([D, COLS], BF16, tag="h3")
                for j in range(COLS // 1024):
                    ps = mmpool.tile([D, 1024], F32, tag="mm")
                    for k in range(2):
                        nc.tensor.matmul(
                            ps[:, ts(k, NCHUNK)],
                            w_t["w3"][:],
                            h2[:, j * 1024 + k * NCHUNK : j * 1024 + (k + 1) * NCHUNK],
                            start=True, stop=True,
                        )
                    if j == 3:
                        nc.scalar.activation(
                            h3[:, ts(j, 1024)], ps[:], AFT.Relu, bias=b_t["b3"][:],
                        )
                    else:
                        nc.vector.tensor_scalar(
                            h3[:, ts(j, 1024)], ps[:],
                            b_t["b3"][:], 0.0, ALU.add, ALU.max,
                        )

                # ---- readout: logits[p, g*16+c] for samples g*128+p ----
                # sample s of agent a lives at column
                # (s//512)*2048 + a*512 + s%512; chunk g covers samples
                # g*128..g*128+127 -> half g//4, offset (g%4)*128.
                lg = lgpool.tile([D, GROUPS * C], F32, tag="lg")
                for g in range(GROUPS):
                    cbase = (g // 4) * 2048 + (g % 4) * D
                    for a in range(A):
                        nc.tensor.matmul(
                            lg[:, ts(g, C)],
                            h3[:, cbase + a * 512 : cbase + a * 512 + D],
                            wz_t[:, ts(a, C)],
                            start=(a == 0), stop=(a == A - 1),
                        )

                # ---- softmax over 16 classes per 16-col group ----
                e = spool.tile([D, GROUPS * C], F32, tag="e")
                nc.scalar.activation(e[:], lg[:], AFT.Exp)
                f = spool.tile([D, GROUPS * C], F32, tag="f")
                nc.vector.tensor_mul(f[:], e[:], eb_t[:])
                s = spool.tile([D, GROUPS], F32, tag="s")
                nc.vector.reduce_sum(
                    s[:], f[:].rearrange("p (g c) -> p g c", c=C),
                    axis=mybir.AxisListType.X,
                )
                r = spool.tile([D, GROUPS], F32, tag="r")
                nc.vector.reciprocal(r[:], s[:])
                p = spool.tile([D, GROUPS * C], F32, tag="p")
                nc.vector.tensor_mul(
                    p[:].rearrange("p (g c) -> p g c", c=C),
                    f[:].rearrange("p (g c) -> p g c", c=C),
                    r[:].unsqueeze(2).broadcast_to([D, GROUPS, C]),
                )

                # ---- store: row st*1024 + g*128 + p ----
                nc.sync.dma_start(
                    out_d[ts(st, ST), :].rearrange("(g p) c -> p g c", p=D),
                    p[:].rearrange("p (g c) -> p g c", c=C),
                )

    nc.compile()
    return nc


def _prep_inputs(inputs):
    """Host-side: fused weights + per-core transposed input shards."""
    f64 = lambda x: np.asarray(x, np.float64)
    enc_w, enc_b = f64(inputs["enc_w"]), f64(inputs["enc_b"])
    fc1_w, fc1_b = f64(inputs["fc1_w"]), f64(inputs["fc1_b"])
    fc2_w, fc2_b = f64(inputs["fc2_w"]), f64(inputs["fc2_b"])
    fc3_w, fc3_b = f64(inputs["fc3_w"]), f64(inputs["fc3_b"])
    cl4_w, cl4_b = f64(inputs["cl4_w"]), f64(inputs["cl4_b"])
    dec_w, dec_b = f64(inputs["dec_w"]), f64(inputs["dec_b"])

    A_ = cl4_w[:D]
    Bm = cl4_w[D:]
    Da = dec_w.reshape(A, D, C)
    Dsum = Da.sum(0)
    Wz = np.concatenate(
        [A_ @ Da[a] + 0.25 * (Bm @ (Dsum - Da[a])) for a in range(A)], axis=1
    )  # [128, 64]
    bias_p = dec_b + cl4_b @ Dsum  # [16]

    import ml_dtypes

    common = {
        "enc_w": np.ascontiguousarray(np.vstack([enc_w, enc_w]), np.float32),
        "w1": np.ascontiguousarray(0.5 * fc1_w, np.float32),
        "w2": np.ascontiguousarray(fc2_w, np.float32),
        "w3": np.ascontiguousarray(fc3_w, np.float32),
        "wz": np.ascontiguousarray(Wz).astype(ml_dtypes.bfloat16),
        "eb": np.tile(np.exp(bias_p).astype(np.float32)[None, :], (D, GROUPS)),
        "b0": (0.5 * enc_b).astype(np.float32).reshape(D, 1),
        "b1": (fc1_b + 0.5 * fc1_w.sum(0)).astype(np.float32).reshape(D, 1),
        "b2": fc2_b.astype(np.float32).reshape(D, 1),
        "b3": fc3_b.astype(np.float32).reshape(D, 1),
    }

    O = np.asarray(inputs["O"], np.float32)  # [B, A, OBS]
    in_maps = []
    for c in range(NCORES):
        oc = O[c * BLOC : (c + 1) * BLOC]                  # [BLOC, A, OBS]
        # ot[h*64+f, st, a*512+s'] = O[st*1024 + h*512 + s', a, f]
        x = oc.reshape(NST, 2, ST // 2, A, OBS)
        ot = np.ascontiguousarray(x.transpose(1, 4, 0, 3, 2)).reshape(
            2 * OBS, NST, COLS // 2
        )
        in_maps.append({"ot": ot, **common})
    return in_maps


def kernel(**inputs):
    if "nc" not in _compiled:
        _compiled["nc"] = _build_bass()
    nc = _compiled["nc"]
    in_maps = _prep_inputs(inputs)
    res = run_bass_kernel_spmd(nc, in_maps, core_ids=list(range(NCORES)))
    return np.concatenate([res.results[i]["probs"] for i in range(NCORES)], axis=0)



# revision 2
# speedup vs baseline: 89.3523x; 89.3523x over previous
"""Trainium2 Bass kernel for nn_CommNetActor — v2.

Same algebra as v1 (tail folded into per-agent readout matrices Wz; sigmoid
rewritten as tanh with the affine folded into fc1), plus:

- Readout runs Wz-stationary: lgT[c,s] = sum_a Wz_a^T H3_chunk, so the
  expensive per-chunk 128-col weight loads of H3 disappear (Wz slices are
  16-col loads). The per-class bias is folded into the Exp activation's
  bias operand (eb constant eliminated). PE transposes put exp(logits)
  back into [sample, class] orientation for a free-dim softmax.
- 7 DMAs total (2 const packs, 4 input, 1 output) instead of 26; the
  output is staged in SBUF [128, 1024] and stored contiguously once,
  host-side de-interleave replaces the scattered 64B-segment stores.
- Elementwise work split across ACT (tanh, exp, 2/4 of fc2) and DVE
  (the other relus + softmax tail) to stay under the PE critical path.

Column layout per 1024-sample super-tile unchanged: sample s of agent a
sits at column (s//512)*2048 + a*512 + s%512; input packed two samples
per column ([128, 2048] per super-tile).
"""

import numpy as np

import concourse.bass as bass
import concourse.mybir as mybir
import concourse.tile as tile
from concourse import bacc
from concourse.bass import ts
from concourse.bass_utils import run_bass_kernel_spmd
from concourse.masks import make_identity

B = 65536
A = 4
OBS = 64
D = 128
C = 16
NCORES = 8
BLOC = B // NCORES
ST = 1024
NST = BLOC // ST
COLS = A * ST
NCHUNK = 512
GROUPS = ST // D            # 8 sample-groups of 128 per super-tile
STPERDMA = 2                # super-tiles per input DMA

F32 = mybir.dt.float32
F32R = mybir.dt.float32r
AFT = mybir.ActivationFunctionType
ALU = mybir.AluOpType

TRUNK_DT = F32R
HDT = mybir.dt.bfloat16

# wpack columns: enc(0:128) w1(128:256) w2(256:384) w3(384:512) wz(512:576)
WCOLS = 576

_compiled = {}


def _build_bass():
    nc = bacc.Bacc()

    ot_d = nc.dram_tensor("ot", [2 * OBS, NST * COLS // 2], TRUNK_DT, kind="ExternalInput")
    wp_d = nc.dram_tensor("wpack", [D, WCOLS], TRUNK_DT, kind="ExternalInput")
    bp_d = nc.dram_tensor("bpack", [D, 8], F32, kind="ExternalInput")
    out_d = nc.dram_tensor("probs", [D, NST * GROUPS * C], F32, kind="ExternalOutput")

    with tile.TileContext(nc) as tc:
        with (
            tc.tile_pool(name="consts", bufs=1) as cpool,
            tc.tile_pool(name="ot", bufs=2) as opool,
            tc.tile_pool(name="acts", bufs=2) as hpool,
            tc.tile_pool(name="soft", bufs=2) as spool,
            tc.tile_pool(name="stage", bufs=1) as stpool,
            tc.tile_pool(name="mm", bufs=6, space="PSUM") as mmpool,
            tc.tile_pool(name="lgT", bufs=1, space="PSUM") as lgpool,
            tc.tile_pool(name="tr", bufs=1, space="PSUM") as trpool,
        ):
            wp_t = cpool.tile([D, WCOLS], TRUNK_DT, name="wp")
            nc.sync.dma_start(wp_t[:], wp_d[:])
            bp_t = cpool.tile([D, 8], F32, name="bp")
            nc.sync.dma_start(bp_t[:], bp_d[:])
            ident = cpool.tile([D, D], F32, name="ident")
            make_identity(nc, ident[:])

            ew = wp_t[:, 0:128]
            w1 = wp_t[:, 128:256]
            w2 = wp_t[:, 256:384]
            w3 = wp_t[:, 384:512]
            wz = wp_t[:, 512:576]
            b0 = bp_t[:, 0:1]
            b1 = bp_t[:, 1:2]
            b2 = bp_t[:, 2:3]
            b3 = bp_t[:, 3:4]
            bsm = bp_t[0:C, 4:5]   # softmax bias (folded dec/cl4 bias), rows 0..15

            stage = stpool.tile([D, NST * GROUPS * C], F32, name="stage")

            for dm in range(NST // STPERDMA):
                ot_t = opool.tile([2 * OBS, STPERDMA * COLS // 2], TRUNK_DT, tag="ot")
                nc.sync.dma_start(
                    ot_t[:], ot_d[:, ts(dm, STPERDMA * COLS // 2)],
                )
                for sst in range(STPERDMA):
                    st = dm * STPERDMA + sst
                    otv = ot_t[:, sst * (COLS // 2):(sst + 1) * (COLS // 2)]

                    def ew_op(engine, dst_ap, ps, b, func):
                        if engine == "A":
                            nc.scalar.activation(dst_ap, ps[:], func, bias=b)
                        elif engine == "G":
                            nc.gpsimd.tensor_scalar(
                                dst_ap, ps[:], b, 0.0, ALU.add, ALU.max,
                            )
                        else:
                            nc.vector.tensor_scalar(
                                dst_ap, ps[:], b, 0.0, ALU.add, ALU.max,
                            )

                    # ---- enc: tanh(0.5 x + 0.5 b); K=64 row-group pairs ----
                    # 512-col chunks; alternate row groups so consecutive
                    # matmuls execute concurrently in the PE array.
                    h0 = hpool.tile([D, COLS], TRUNK_DT, tag="h0")
                    for cb in range(4):
                        for hh in range(2):
                            ps = mmpool.tile([D, NCHUNK], F32, tag="mm")
                            nc.tensor.matmul(
                                ps[:],
                                ew[64 * hh: 64 * (hh + 1), :],
                                otv[64 * hh: 64 * (hh + 1), ts(cb, NCHUNK)],
                                start=True, stop=True,
                            )
                            nc.scalar.activation(
                                h0[:, hh * 2048 + cb * NCHUNK:
                                   hh * 2048 + (cb + 1) * NCHUNK],
                                ps[:], AFT.Tanh, bias=b0, scale=0.5,
                            )

                    # ---- fc trunk; 512-col chunks; EW split ACT/DVE/Pool ----
                    def fc(dst, src, w, b, engines):
                        for j in range(8):
                            ps = mmpool.tile([D, NCHUNK], F32, tag="mm")
                            nc.tensor.matmul(
                                ps[:], w, src[:, ts(j, NCHUNK)],
                                start=True, stop=True,
                            )
                            ew_op(engines[j], dst[:, ts(j, NCHUNK)], ps, b,
                                  AFT.Relu)

                    h1 = hpool.tile([D, COLS], TRUNK_DT, tag="h1")
                    fc(h1, h0, w1, b1, "VVVVVVVV")
                    h2 = hpool.tile([D, COLS], TRUNK_DT, tag="h2")
                    fc(h2, h1, w2, b2, "AAVVAAVV")
                    h3 = hpool.tile([D, COLS], TRUNK_DT, tag="h3")
                    fc(h3, h2, w3, b3, "AAVVAAVV")

                    # ---- readout: Wz-stationary, class-major logits ----
                    for h in range(2):
                        lgT = lgpool.tile([C, NCHUNK], F32, tag="lgT")
                        for a in range(A):
                            nc.tensor.matmul(
                                lgT[:],
                                wz[:, ts(a, C)],
                                h3[:, h * 2048 + a * NCHUNK:
                                   h * 2048 + (a + 1) * NCHUNK],
                                start=(a == 0), stop=(a == A - 1),
                            )
                        # exp(logits + bias) straight out of PSUM
                        e = spool.tile([C, NCHUNK], F32, tag="e")
                        nc.scalar.activation(e[:], lgT[:], AFT.Exp, bias=bsm)
                        # back to [sample, class] via PE transpose
                        tr = trpool.tile([D, 4 * C], F32, tag="tr")
                        for gg in range(4):
                            nc.tensor.transpose(
                                tr[:, ts(gg, C)], e[:, ts(gg, D)], ident[0:C, 0:C],
                            )
                        # softmax tail in free dim
                        s4 = spool.tile([D, 4], F32, tag="s4")
                        nc.vector.reduce_sum(
                            s4[:], tr[:].rearrange("p (g c) -> p g c", c=C),
                            axis=mybir.AxisListType.X,
                        )
                        r4 = spool.tile([D, 4], F32, tag="r4")
                        nc.vector.reciprocal(r4[:], s4[:])
                        nc.vector.tensor_mul(
                            stage[:, st * 128 + h * 64: st * 128 + (h + 1) * 64]
                            .rearrange("p (g c) -> p g c", c=C),
                            tr[:].rearrange("p (g c) -> p g c", c=C),
                            r4[:].unsqueeze(2).broadcast_to([D, 4, C]),
                        )

            nc.sync.dma_start(out_d[:], stage[:])

    nc.compile()
    return nc


def _prep_inputs(inputs):
    """Host-side: fused weights + per-core transposed input shards."""
    f64 = lambda x: np.asarray(x, np.float64)
    enc_w, enc_b = f64(inputs["enc_w"]), f64(inputs["enc_b"])
    fc1_w, fc1_b = f64(inputs["fc1_w"]), f64(inputs["fc1_b"])
    fc2_w, fc2_b = f64(inputs["fc2_w"]), f64(inputs["fc2_b"])
    fc3_w, fc3_b = f64(inputs["fc3_w"]), f64(inputs["fc3_b"])
    cl4_w, cl4_b = f64(inputs["cl4_w"]), f64(inputs["cl4_b"])
    dec_w, dec_b = f64(inputs["dec_w"]), f64(inputs["dec_b"])

    A_ = cl4_w[:D]
    Bm = cl4_w[D:]
    Da = dec_w.reshape(A, D, C)
    Dsum = Da.sum(0)
    Wz = np.concatenate(
        [A_ @ Da[a] + 0.25 * (Bm @ (Dsum - Da[a])) for a in range(A)], axis=1
    )  # [128, 64]
    bias_p = dec_b + cl4_b @ Dsum  # [16]

    wpack = np.concatenate(
        [np.vstack([enc_w, enc_w]), 0.5 * fc1_w, fc2_w, fc3_w, Wz], axis=1
    ).astype(np.float32)
    bpack = np.zeros((D, 8), np.float32)
    bpack[:, 0] = 0.5 * enc_b
    bpack[:, 1] = fc1_b + 0.5 * fc1_w.sum(0)
    bpack[:, 2] = fc2_b
    bpack[:, 3] = fc3_b
    bpack[:C, 4] = bias_p

    O = np.asarray(inputs["O"], np.float32)  # [B, A, OBS]
    in_maps = []
    for c in range(NCORES):
        oc = O[c * BLOC: (c + 1) * BLOC]
        x = oc.reshape(NST, 2, ST // 2, A, OBS)
        ot = np.ascontiguousarray(x.transpose(1, 4, 0, 3, 2)).reshape(
            2 * OBS, NST * COLS // 2
        )
        in_maps.append({"ot": ot, "wpack": wpack, "bpack": bpack})
    return in_maps


def _unstage(arr):
    """[128, NST*GROUPS*16] staging -> [BLOC, 16].

    staging col = st*128 + h*64 + gg*16 + c holds sample
    st*1024 + h*512 + gg*128 + p."""
    x = arr.reshape(D, NST, 2, 4, C)
    return np.ascontiguousarray(x.transpose(1, 2, 3, 0, 4)).reshape(BLOC, C)


def kernel(**inputs):
    if "nc" not in _compiled:
        _compiled["nc"] = _build_bass()
    nc = _compiled["nc"]
    in_maps = _prep_inputs(inputs)
    res = run_bass_kernel_spmd(nc, in_maps, core_ids=list(range(NCORES)))
    return np.concatenate(
        [_unstage(res.results[i]["probs"]) for i in range(NCORES)], axis=0
    )


# revision 4
# speedup vs baseline: 121.4367x; 1.3591x over previous
"""Trainium2 Bass kernel for nn_CommNetActor — v2.

Same algebra as v1 (tail folded into per-agent readout matrices Wz; sigmoid
rewritten as tanh with the affine folded into fc1), plus:

- Full-bf16 trunk (input, weights, activations; f32 PSUM accumulate):
  the BIR verifier forbids mixing 32-bit and non-32-bit matmul inputs,
  so bf16 anywhere means bf16 everywhere in the trunk. Enables FWL on
  weight loads, halves activation SBUF and input upload; measured rel
  err 1.6e-4 end-to-end.

- Readout runs Wz-stationary: lgT[c,s] = sum_a Wz_a^T H3_chunk, so the
  expensive per-chunk 128-col weight loads of H3 disappear (Wz slices are
  16-col loads). The per-class bias is folded into the Exp activation's
  bias operand (eb constant eliminated). PE transposes put exp(logits)
  back into [sample, class] orientation for a free-dim softmax.
- 7 DMAs total (2 const packs, 4 input, 1 output) instead of 26; the
  output is staged in SBUF [128, 1024] and stored contiguously once,
  host-side de-interleave replaces the scattered 64B-segment stores.
- Elementwise work in 512-col chunks through 6 single-bank PSUM buffers
  (6 chunk-pipelines in flight), split across ACT (tanh, exp, half of
  fc2/fc3) and DVE (the rest + softmax tail). GPSIMD cannot read PSUM
  on TRN2 hardware (the BIR verifier rejects it), so it gets no
  PSUM->SBUF bias+relu work despite being idle.

Column layout per 1024-sample super-tile unchanged: sample s of agent a
sits at column (s//512)*2048 + a*512 + s%512; input packed two samples
per column ([128, 2048] per super-tile).
"""

import numpy as np

import concourse.bass as bass
import concourse.mybir as mybir
import concourse.tile as tile
from concourse import bacc
from concourse.bass import ts
from concourse.bass_utils import run_bass_kernel_spmd
from concourse.masks import make_identity

B = 65536
A = 4
OBS = 64
D = 128
C = 16
NCORES = 8
BLOC = B // NCORES
ST = 1024
NST = BLOC // ST
COLS = A * ST
NCHUNK = 512
GROUPS = ST // D            # 8 sample-groups of 128 per super-tile
STPERDMA = 2                # super-tiles per input DMA

F32 = mybir.dt.float32
F32R = mybir.dt.float32r
AFT = mybir.ActivationFunctionType
ALU = mybir.AluOpType

TRUNK_DT = mybir.dt.bfloat16
HDT = mybir.dt.bfloat16

# wpack columns: enc(0:128) w1(128:256) w2(256:384) w3(384:512) wz(512:576)
WCOLS = 576

_compiled = {}


def _build_bass():
    nc = bacc.Bacc()

    ot_d = nc.dram_tensor("ot", [2 * OBS, NST * COLS // 2], TRUNK_DT, kind="ExternalInput")
    wp_d = nc.dram_tensor("wpack", [D, WCOLS], TRUNK_DT, kind="ExternalInput")
    bp_d = nc.dram_tensor("bpack", [D, 8], F32, kind="ExternalInput")
    out_d = nc.dram_tensor("probs", [D, NST * GROUPS * C], F32, kind="ExternalOutput")

    with tile.TileContext(nc) as tc:
        with (
            tc.tile_pool(name="consts", bufs=1) as cpool,
            tc.tile_pool(name="ot", bufs=2) as opool,
            tc.tile_pool(name="acts", bufs=2) as hpool,
            tc.tile_pool(name="soft", bufs=2) as spool,
            tc.tile_pool(name="stage", bufs=1) as stpool,
            tc.tile_pool(name="mm", bufs=6, space="PSUM") as mmpool,
            tc.tile_pool(name="lgT", bufs=1, space="PSUM") as lgpool,
            tc.tile_pool(name="tr", bufs=1, space="PSUM") as trpool,
        ):
            wp_t = cpool.tile([D, WCOLS], TRUNK_DT, name="wp")
            nc.sync.dma_start(wp_t[:], wp_d[:])
            bp_t = cpool.tile([D, 8], F32, name="bp")
            nc.sync.dma_start(bp_t[:], bp_d[:])
            ident = cpool.tile([D, D], F32, name="ident")
            make_identity(nc, ident[:])

            ew = wp_t[:, 0:128]
            w1 = wp_t[:, 128:256]
            w2 = wp_t[:, 256:384]
            w3 = wp_t[:, 384:512]
            wz = wp_t[:, 512:576]
            b0 = bp_t[:, 0:1]
            b1 = bp_t[:, 1:2]
            b2 = bp_t[:, 2:3]
            b3 = bp_t[:, 3:4]
            bsm = bp_t[0:C, 4:5]   # softmax bias (folded dec/cl4 bias), rows 0..15

            stage = stpool.tile([D, NST * GROUPS * C], F32, name="stage")

            for dm in range(NST // STPERDMA):
                ot_t = opool.tile([2 * OBS, STPERDMA * COLS // 2], TRUNK_DT, tag="ot")
                nc.sync.dma_start(
                    ot_t[:], ot_d[:, ts(dm, STPERDMA * COLS // 2)],
                )
                for sst in range(STPERDMA):
                    st = dm * STPERDMA + sst
                    otv = ot_t[:, sst * (COLS // 2):(sst + 1) * (COLS // 2)]

                    def ew_op(engine, dst_ap, ps, b, func):
                        if engine == "A":
                            nc.scalar.activation(dst_ap, ps[:], func, bias=b)
                        elif engine == "G":
                            nc.gpsimd.tensor_scalar(
                                dst_ap, ps[:], b, 0.0, ALU.add, ALU.max,
                            )
                        else:
                            nc.vector.tensor_scalar(
                                dst_ap, ps[:], b, 0.0, ALU.add, ALU.max,
                            )

                    # ---- enc: tanh(0.5 x + 0.5 b); K=64 row-group pairs ----
                    # 512-col chunks; alternate row groups so consecutive
                    # matmuls execute concurrently in the PE array.
                    h0 = hpool.tile([D, COLS], HDT, tag="h0")
                    for cb in range(4):
                        for hh in range(2):
                            ps = mmpool.tile([D, NCHUNK], F32, tag="mm")
                            nc.tensor.matmul(
                                ps[:],
                                ew[64 * hh: 64 * (hh + 1), :],
                                otv[64 * hh: 64 * (hh + 1), ts(cb, NCHUNK)],
                                start=True, stop=True,
                            )
                            nc.scalar.activation(
                                h0[:, hh * 2048 + cb * NCHUNK:
                                   hh * 2048 + (cb + 1) * NCHUNK],
                                ps[:], AFT.Tanh, bias=b0, scale=0.5,
                            )

                    # ---- fc trunk; 512-col chunks; EW split ACT/DVE/Pool ----
                    def fc(dst, src, w, b, engines):
                        for j in range(8):
                            ps = mmpool.tile([D, NCHUNK], F32, tag="mm")
                            nc.tensor.matmul(
                                ps[:], w, src[:, ts(j, NCHUNK)],
                                start=True, stop=True,
                            )
                            ew_op(engines[j], dst[:, ts(j, NCHUNK)], ps, b,
                                  AFT.Relu)

                    h1 = hpool.tile([D, COLS], HDT, tag="h1")
                    fc(h1, h0, w1, b1, "VVVVVVVV")
                    h2 = hpool.tile([D, COLS], HDT, tag="h2")
                    fc(h2, h1, w2, b2, "AAVVAAVV")
                    h3 = hpool.tile([D, COLS], HDT, tag="h3")
                    fc(h3, h2, w3, b3, "AAVVAAVV")

                    # ---- readout: Wz-stationary, class-major logits ----
                    for h in range(2):
                        lgT = lgpool.tile([C, NCHUNK], F32, tag="lgT")
                        for a in range(A):
                            nc.tensor.matmul(
                                lgT[:],
                                wz[:, ts(a, C)],
                                h3[:, h * 2048 + a * NCHUNK:
                                   h * 2048 + (a + 1) * NCHUNK],
                                start=(a == 0), stop=(a == A - 1),
                            )
                        # exp(logits + bias) straight out of PSUM
                        e = spool.tile([C, NCHUNK], F32, tag="e")
                        nc.scalar.activation(e[:], lgT[:], AFT.Exp, bias=bsm)
                        # back to [sample, class] via PE transpose
                        tr = trpool.tile([D, 4 * C], F32, tag="tr")
                        for gg in range(4):
                            nc.tensor.transpose(
                                tr[:, ts(gg, C)], e[:, ts(gg, D)], ident[0:C, 0:C],
                            )
                        # softmax tail in free dim
                        s4 = spool.tile([D, 4], F32, tag="s4")
                        nc.vector.reduce_sum(
                            s4[:], tr[:].rearrange("p (g c) -> p g c", c=C),
                            axis=mybir.AxisListType.X,
                        )
                        r4 = spool.tile([D, 4], F32, tag="r4")
                        nc.vector.reciprocal(r4[:], s4[:])
                        nc.vector.tensor_mul(
                            stage[:, st * 128 + h * 64: st * 128 + (h + 1) * 64]
                            .rearrange("p (g c) -> p g c", c=C),
                            tr[:].rearrange("p (g c) -> p g c", c=C),
                            r4[:].unsqueeze(2).broadcast_to([D, 4, C]),
                        )

            nc.sync.dma_start(out_d[:], stage[:])

    nc.compile()
    return nc


def _prep_inputs(inputs):
    """Host-side: fused weights + per-core transposed input shards."""
    f64 = lambda x: np.asarray(x, np.float64)
    enc_w, enc_b = f64(inputs["enc_w"]), f64(inputs["enc_b"])
    fc1_w, fc1_b = f64(inputs["fc1_w"]), f64(inputs["fc1_b"])
    fc2_w, fc2_b = f64(inputs["fc2_w"]), f64(inputs["fc2_b"])
    fc3_w, fc3_b = f64(inputs["fc3_w"]), f64(inputs["fc3_b"])
    cl4_w, cl4_b = f64(inputs["cl4_w"]), f64(inputs["cl4_b"])
    dec_w, dec_b = f64(inputs["dec_w"]), f64(inputs["dec_b"])

    A_ = cl4_w[:D]
    Bm = cl4_w[D:]
    Da = dec_w.reshape(A, D, C)
    Dsum = Da.sum(0)
    Wz = np.concatenate(
        [A_ @ Da[a] + 0.25 * (Bm @ (Dsum - Da[a])) for a in range(A)], axis=1
    )  # [128, 64]
    bias_p = dec_b + cl4_b @ Dsum  # [16]

    import ml_dtypes
    wpack = np.concatenate(
        [np.vstack([enc_w, enc_w]), 0.5 * fc1_w, fc2_w, fc3_w, Wz], axis=1
    ).astype(ml_dtypes.bfloat16)
    bpack = np.zeros((D, 8), np.float32)
    bpack[:, 0] = 0.5 * enc_b
    bpack[:, 1] = fc1_b + 0.5 * fc1_w.sum(0)
    bpack[:, 2] = fc2_b
    bpack[:, 3] = fc3_b
    bpack[:C, 4] = bias_p

    O = np.asarray(inputs["O"], np.float32)  # [B, A, OBS]
    in_maps = []
    for c in range(NCORES):
        oc = O[c * BLOC: (c + 1) * BLOC]
        x = oc.reshape(NST, 2, ST // 2, A, OBS)
        ot = np.ascontiguousarray(
            x.transpose(1, 4, 0, 3, 2).astype(ml_dtypes.bfloat16)
        ).reshape(2 * OBS, NST * COLS // 2)
        in_maps.append({"ot": ot, "wpack": wpack, "bpack": bpack})
    return in_maps


def _unstage(arr):
    """[128, NST*GROUPS*16] staging -> [BLOC, 16].

    staging col = st*128 + h*64 + gg*16 + c holds sample
    st*1024 + h*512 + gg*128 + p."""
    x = arr.reshape(D, NST, 2, 4, C)
    return np.ascontiguousarray(x.transpose(1, 2, 3, 0, 4)).reshape(BLOC, C)


def kernel(**inputs):
    if "nc" not in _compiled:
        _compiled["nc"] = _build_bass()
    nc = _compiled["nc"]
    in_maps = _prep_inputs(inputs)
    res = run_bass_kernel_spmd(nc, in_maps, core_ids=list(range(NCORES)))
    return np.concatenate(
        [_unstage(res.results[i]["probs"]) for i in range(NCORES)], axis=0
    )
